# revision 1
# baseline (speedup 1.0000x reference)
"""Autoformer encoder layer on 8 Trainium2 NeuronCores (Bass/Tile).

Data-parallel over batch: each of the 8 cores processes 4 of the 32 batches.
Everything runs on-device in one NEFF, including the cross-core reduction for
the global top-k delay selection (AllReduce of the batch-summed correlation)
and the data-dependent delay rolls (dynamic-offset SBUF reads).

Math notes
----------
The reference only ever uses two reductions of the full per-(head,channel)
autocorrelation:
  * mean_value[b, l] = mean over channels of irfft(rfft(q)*conj(rfft(k)))
    == (1/D) * sum_j <q[j+l], k[j]>  (circular cross-correlation theorem).
    Computed here fully in the TIME domain: with Wkq := Wk @ Wq^T and
    y := x @ Wkq, corr[l] = sum_j <x[j+l], y[j]> (the q/k biases only add a
    per-batch constant to corr, which drops out of both the global argsort
    and the per-batch softmax, so they are omitted). The correlation itself
    is 36 PE matmuls per (batch, lag-block): contraction over channels with
    a doubled-x buffer providing the circular shifts. Each [128, 512] PSUM
    block holds lags on anti-diagonals (lag = l0 + col - row); a diagonal
    DMA (per-partition -1 element skew) realigns them into columns, and a
    ones-vector matmul (scaled 1/D) does the partition sum.
  * agg = sum_i softmax(w)_i * roll(v, -d_i)  -> rolls commute with the
    output projection, so v@Wo is computed directly with folded weights
    Wvo = Wv @ Wo and rolled instead (7 dynamic-slice MACs per channel chunk).
The moving-average decomposition runs as a cumsum scan along the free axis in
channel-major layout, so the whole residual/FFN pipeline needs no on-device
transposes: x is fed pre-transposed as [B, D, L] and the output is returned
transposed, undone on the host.
"""

import os
from contextlib import ExitStack

import numpy as np

import concourse.bass as bass
import concourse.bacc as bacc
import concourse.mybir as mybir
from concourse import tile
from concourse.bass_utils import run_bass_kernel_spmd
from concourse.ordered_set import OrderedSet

F32 = mybir.dt.float32
F32R = mybir.dt.float32r
F16 = mybir.dt.float16
U32 = mybir.dt.uint32
AX = mybir.AxisListType
OP = mybir.AluOpType
AF = mybir.ActivationFunctionType
DVE = mybir.EngineType.DVE

B, L, D, DFF = 32, 1536, 512, 2048
KMA = 25              # moving-average window
PAD = (KMA - 1) // 2  # 12
TOPK = 7              # int(1 * log(1536))
N_CORES = 8
BC = B // N_CORES     # batches per core
NLC = L // 128        # 12 l-chunks (also j-tiles)
NDC = D // 128        # 4 channel chunks
NFC = DFF // 128      # 16 ffn chunks
NLB = L // 512        # 3 l-blocks of 512
RW = 512 + 127        # realigned block width (lags l0-127 .. l0+511)


def _host_consts():
    # moving-average edge coefficients, pre-negated for fused (coef*edge)+rest
    coefL = np.tile((-(PAD - np.arange(PAD)) / KMA).astype(np.float32), (128, 1))
    coefR = np.tile((-(np.arange(PAD) + 1) / KMA).astype(np.float32), (128, 1))
    return coefL, coefR


def build(dbg=False):
    phases = int(os.environ.get("KPHASES", "2"))
    p1b = int(os.environ.get("KP1B", str(BC)))
    kreps = int(os.environ.get("KREPS", "1"))
    kar = int(os.environ.get("KAR", "1"))
    nc = bacc.Bacc("TRN2", target_bir_lowering=False, debug=False, num_devices=N_CORES)

    xT = nc.dram_tensor("xT", [BC, D, L], F32R, kind="ExternalInput")
    # fp16 copy of x for phase 2 (rolls at 2x DVE rate, halved SBUF)
    xTb_d = nc.dram_tensor("xTb", [BC, D, L], F16, kind="ExternalInput")
    Wkq_d = nc.dram_tensor("Wkq", [D, D], F32R, kind="ExternalInput")
    Wvo_d = nc.dram_tensor("Wvo", [D, D], F16, kind="ExternalInput")
    W1_d = nc.dram_tensor("W1", [D, DFF], F16, kind="ExternalInput")
    W2_d = nc.dram_tensor("W2", [DFF, D], F16, kind="ExternalInput")
    # channel-major biases prepacked host-side as [128, nchunks]
    bvo_d = nc.dram_tensor("bvo", [128, NDC], F32, kind="ExternalInput")
    b1_d = nc.dram_tensor("b1", [128, NFC], F32, kind="ExternalInput")
    b2_d = nc.dram_tensor("b2", [128, NDC], F32, kind="ExternalInput")
    coefL_d = nc.dram_tensor("coefL", [128, PAD], F32, kind="ExternalInput")
    coefR_d = nc.dram_tensor("coefR", [128, PAD], F32, kind="ExternalInput")

    resT = nc.dram_tensor("resT", [BC, D, L], F32, kind="ExternalOutput")
    if dbg:
        mv_dbg = nc.dram_tensor("mv_dbg", [5, L], F32, kind="ExternalOutput")
        idx_dbg = nc.dram_tensor("idx_dbg", [1, 8], U32, kind="ExternalOutput")
        w_dbg = nc.dram_tensor("w_dbg", [BC, TOPK], F32, kind="ExternalOutput")

    with tile.TileContext(nc) as tc, ExitStack() as stack:
        pp = stack.enter_context(tc.tile_pool(name="persist", bufs=1))
        dram = stack.enter_context(tc.tile_pool(name="dram", bufs=1, space="DRAM"))

        # ---- persistent biases / constants -------------------------------
        bvoT = pp.tile([128, NDC], F32, tag="bvoT")
        b1T = pp.tile([128, NFC], F32, tag="b1T")
        b2T = pp.tile([128, NDC], F32, tag="b2T")
        nc.sync.dma_start(out=bvoT[:, :], in_=bvo_d[:, :])
        nc.sync.dma_start(out=b1T[:, :], in_=b1_d[:, :])
        nc.sync.dma_start(out=b2T[:, :], in_=b2_d[:, :])

        coefL_sb = pp.tile([128, PAD], F32, tag="coefL")
        coefR_sb = pp.tile([128, PAD], F32, tag="coefR")
        nc.sync.dma_start(out=coefL_sb[:, :], in_=coefL_d[:, :])
        nc.sync.dma_start(out=coefR_sb[:, :], in_=coefR_d[:, :])

        # ones vectors for partition sums (1/D folds the channel mean);
        # these feed plain-fp32 matmuls (tiny N) so no f32r rounding rules
        onesD = pp.tile([128, 1], F32, tag="onesD")
        nc.vector.memset(onesD[:, :], 1.0 / D)
        onesB = pp.tile([BC, 1], F32, tag="onesB")
        nc.vector.memset(onesB[:, :], 1.0)

        mv_sb = pp.tile([5, L], F32, tag="mv")  # rows 0-3: batches, 4: batchsum
        idx_sb = pp.tile([1, 8], U32, tag="idx")
        wbc = pp.tile([128, BC * TOPK], F32, tag="wbc")
        wv = pp.tile([BC, TOPK], F32, tag="wv")
        radd = pp.tile([1, NLB * 127], F32, tag="radd")  # wrap-around lag pieces

        for _rep in range(kreps):
            # ============ PHASE 1: y = x@Wkq, time-domain correlation ========
            with ExitStack() as p1stack:
                p1c = p1stack.enter_context(tc.tile_pool(name="p1c", bufs=1))
                wkq_sb = []
                for c in range(NDC):
                    t = p1c.tile([128, D], F32R, tag=f"wkq{c}")
                    nc.sync.dma_start(out=t[:, :], in_=Wkq_d[128 * c : 128 * (c + 1), :])
                    wkq_sb.append(t)

                pbatch = ExitStack()
                p1x = pbatch.enter_context(tc.tile_pool(name="p1x", bufs=2))
                p1y = pbatch.enter_context(tc.tile_pool(name="p1y", bufs=2))
                p1s = pbatch.enter_context(tc.tile_pool(name="p1s", bufs=2))
                psy = pbatch.enter_context(tc.tile_pool(name="psumy", bufs=2, space="PSUM"))
                psS = pbatch.enter_context(tc.tile_pool(name="psumS", bufs=2, space="PSUM"))
                psr = pbatch.enter_context(tc.tile_pool(name="psumr", bufs=1, space="PSUM"))

                for b in range(p1b):
                    mvrow = p1s.tile([1, L], F32, tag="mvrow", bufs=2)
                    # doubled-x buffer per channel chunk: [x | x] for circular reads
                    x2d = [p1x.tile([128, 2 * L], F32R, tag=f"x2d{c}", name=f"x2d{c}")
                           for c in range(NDC)]
                    for c in range(NDC):
                        nc.sync.dma_start(
                            out=x2d[c][:, 0:L], in_=xT[b, 128 * c : 128 * (c + 1), :])
                        nc.sync.dma_start(
                            out=x2d[c][:, L : 2 * L], in_=xT[b, 128 * c : 128 * (c + 1), :])

                    # y = x @ Wkq in channel-major: y[cout, l] = sum_cin Wkq[cin, cout] x[cin, l]
                    y_sb = [p1y.tile([128, L], F32R, tag=f"y{c}", name=f"y{c}")
                            for c in range(NDC)]
                    for co in range(NDC):
                        for nb in range(NLB):
                            py = psy.tile([128, 512], F32, tag="py")
                            for ci in range(NDC):
                                nc.tensor.matmul(
                                    py[:, :],
                                    wkq_sb[ci][:, 128 * co : 128 * (co + 1)],
                                    x2d[ci][:, 512 * nb : 512 * (nb + 1)],
                                    start=(ci == 0), stop=(ci == NDC - 1))
                            nc.scalar.copy(y_sb[co][:, 512 * nb : 512 * (nb + 1)], py[:, :])

                    # correlation blocks: S[j_loc, n] = sum_{jt,c} y[c, j0+j_loc] x[c, j0+l0+n]
                    # lag of element (j_loc, n) is l0 + n - j_loc  (constant across jt)
                    for bi in range(NLB):
                        l0 = 512 * bi
                        S = psS.tile([128, 512], F32, tag="S")
                        first = True
                        for jt in range(NLC):
                            for ci in range(NDC):
                                nc.tensor.matmul(
                                    S[:, :],
                                    y_sb[ci][:, 128 * jt : 128 * (jt + 1)],
                                    x2d[ci][:, 128 * jt + l0 : 128 * jt + l0 + 512],
                                    start=first,
                                    stop=(jt == NLC - 1 and ci == NDC - 1))
                                first = False
                        Stmp = p1s.tile([128, 512], F32, tag="Stmp")
                        nc.scalar.copy(Stmp[:, :], S[:, :])
                        # realign anti-diagonals into columns: row j -> cols 127-j ..
                        Wt = p1s.tile([128, RW], F32, tag="Wt")
                        nc.vector.memset(Wt[:, :], 0.0)
                        diag = bass.AP(Wt.tensor, 127, [[RW - 1, 128], [1, 512]])
                        nc.sync.dma_start(out=diag, in_=Stmp[:, :])
                        # partition sum via ones-matmul (scaled 1/D)
                        rp = psr.tile([1, 512], F32, tag="rp")
                        rp2 = psr.tile([1, 127], F32, tag="rp2")
                        nc.tensor.matmul(rp[:, :], onesD[:, 0:1], Wt[:, 0:512])
                        nc.tensor.matmul(rp2[:, :], onesD[:, 0:1], Wt[:, 512:RW])
                        # assemble on partition 0: lags [l0, l0+385) from
                        # rp[127:512], [l0+385, l0+512) from rp2, wrap saved
                        nc.vector.tensor_copy(
                            mvrow[0:1, l0 : l0 + 385], rp[0:1, 127:512])
                        nc.vector.tensor_copy(
                            mvrow[0:1, l0 + 385 : l0 + 512], rp2[0:1, 0:127])
                        nc.vector.tensor_copy(
                            radd[0:1, 127 * bi : 127 * (bi + 1)], rp[0:1, 0:127])
                    # wrap-around adds: block l0 lags [l0-127, l0) mod L
                    for bi in range(NLB):
                        lo = (512 * bi - 127) % L
                        nc.vector.tensor_add(
                            mvrow[0:1, lo : lo + 127],
                            mvrow[0:1, lo : lo + 127],
                            radd[0:1, 127 * bi : 127 * (bi + 1)])
                    # DVE lanes can't shift partitions: DMA row to partition b
                    nc.sync.dma_start(out=mv_sb[b : b + 1, :], in_=mvrow[0:1, :])

                # batch-sum row 4 via ones-matmul over partitions 0..3
                if phases >= 0 and p1b == BC:
                    mvr4 = p1s.tile([1, L], F32, tag="mvrow", bufs=2)
                    for nb in range(NLB):
                        rs = psr.tile([1, 512], F32, tag="rs")
                        nc.tensor.matmul(rs[:, :], onesB[:, 0:1],
                                         mv_sb[0:BC, 512 * nb : 512 * (nb + 1)])
                        nc.vector.tensor_copy(
                            mvr4[0:1, 512 * nb : 512 * (nb + 1)], rs[0:1, :])
                    nc.sync.dma_start(out=mv_sb[4:5, :], in_=mvr4[0:1, :])

                pbatch.close()

            if phases < 0:
                nc.vector.memset(mv_sb[:, :], 0.0)
                nc.vector.memset(idx_sb[:, :], 0)
                nc.vector.memset(wbc[:, :], 0.0)
                nc.vector.memset(wv[:, :], 0.0)

            # AllReduce the batch-summed correlation -> global over all 32 batches
            do_ar = phases >= 0 and kar != 0
            cc_in = dram.tile([1, L], F32)
            cc_out = dram.tile([1, L], F32)
            mvg = pp.tile([1, L], F32, tag="mvg")
            max8 = pp.tile([1, 8], F32, tag="max8")
            if phases >= 0 and not do_ar:
                nc.vector.memset(idx_sb[:, :], 0)
            if do_ar:
                nc.sync.dma_start(out=cc_in[:, :], in_=mv_sb[4:5, :])
                nc.gpsimd.collective_compute(
                    "AllReduce",
                    OP.add,
                    replica_groups=[list(range(N_CORES))],
                    ins=[cc_in[:, :].opt()],
                    outs=[cc_out[:, :].opt()],
                )
                nc.sync.dma_start(out=mvg[:, :], in_=cc_out[:, :])
                nc.vector.max(out=max8[:, :], in_=mvg[:, :])
                nc.vector.max_index(out=idx_sb[:, :], in_max=max8[:, :], in_values=mvg[:, :])

            ntk = range(TOPK) if phases >= 1 else range(0)
            dvals = [
                nc.values_load(
                    idx_sb[0:1, i : i + 1],
                    engines=OrderedSet([DVE]),
                    min_val=0,
                    max_val=L - 1,
                    skip_runtime_bounds_check=True,
                )
                for i in ntk
            ]

            # per-batch weights at the selected delays + softmax, then broadcast
            if 0 <= phases < 1:
                nc.vector.memset(wv[:, :], 0.0)
            for i in ntk:
                nc.vector.tensor_copy(wv[:, i : i + 1], mv_sb[0:BC, bass.ds(dvals[i], 1)])
            wred = pp.tile([BC, 2], F32, tag="wred")
            if phases < 1:
                nc.vector.memset(wbc[:, :], 0.0)
            if phases >= 1:
                nc.vector.reduce_max(wred[:, 0:1], wv[:, :], axis=AX.X)
                wexp = pp.tile([BC, TOPK], F32, tag="wexp")
                nc.vector.tensor_scalar(
                    wexp[:, :], wv[:, :], wred[:, 0:1], None, op0=OP.subtract)
                nc.scalar.activation(wexp[:, :], wexp[:, :], AF.Exp)
                nc.vector.reduce_sum(wred[:, 1:2], wexp[:, :], axis=AX.X)
                nc.vector.reciprocal(wred[:, 1:2], wred[:, 1:2])
                nc.vector.tensor_scalar(
                    wexp[:, :], wexp[:, :], wred[:, 1:2], None, op0=OP.mult)
                w_dram = dram.tile([BC, TOPK], F32)
                nc.sync.dma_start(out=w_dram[:, :], in_=wexp[:, :])
                wflat = pp.tile([1, BC * TOPK], F32, tag="wflat")
                nc.sync.dma_start(out=wflat[:, :], in_=w_dram[:, :])
                nc.gpsimd.partition_broadcast(wbc[:, :], wflat[0:1, :])

            if dbg:
                nc.sync.dma_start(out=mv_dbg[:, :], in_=mv_sb[:, :])
                nc.sync.dma_start(out=idx_dbg[:, :], in_=idx_sb[:, :])
                nc.sync.dma_start(out=w_dbg[:, :], in_=wexp[:, :] if phases >= 1 else wv[:, :])

            # ================= PHASE 2: rolls, decomp, FFN, decomp =============
            def ma_seasonal(pool, dst, src):
                """dst = src - moving_avg(src) along the free axis (edge-replicated).

                src must be F32-readable; dst may be F32 or F32R."""
                cs1 = pool.tile([128, L + 1], F32, tag="cs1", bufs=1)
                nc.vector.memset(cs1[:, 0:1], 0.0)
                nc.vector.tensor_tensor_scan(
                    cs1[:, 1 : L + 1], src[:, :], src[:, :], 0.0,
                    op0=OP.add, op1=OP.bypass)
                dif = pool.tile([128, L - 2 * PAD], F32, tag="dif", bufs=1)
                nc.vector.tensor_sub(
                    dif[:, :], cs1[:, 2 * PAD + 1 : L + 1], cs1[:, 0 : L - 2 * PAD])
                nc.vector.scalar_tensor_tensor(
                    out=dst[:, PAD : L - PAD], in0=dif[:, :], scalar=-1.0 / KMA,
                    in1=src[:, PAD : L - PAD], op0=OP.mult, op1=OP.add)
                # left edge: s[l] = x[l] - cs1[l+PAD+1]/K - (PAD-l)/K * x[0]
                nc.vector.scalar_tensor_tensor(
                    out=dst[:, 0:PAD], in0=cs1[:, PAD + 1 : 2 * PAD + 1],
                    scalar=-1.0 / KMA, in1=src[:, 0:PAD], op0=OP.mult, op1=OP.add)
                nc.vector.scalar_tensor_tensor(
                    out=dst[:, 0:PAD], in0=coefL_sb[:, :], scalar=src[:, 0:1],
                    in1=dst[:, 0:PAD], op0=OP.mult, op1=OP.add)
                # right edge: s[l] = x[l] - (stot - cs1[l-PAD])/K - (l-L+PAD+1)/K * x[L-1]
                e2 = pool.tile([128, PAD], F32, tag="e2", bufs=1)
                nc.vector.tensor_scalar(
                    e2[:, :], cs1[:, L - 2 * PAD : L - PAD], cs1[:, L : L + 1],
                    1.0 / KMA, op0=OP.subtract, op1=OP.mult)
                nc.vector.tensor_add(
                    dst[:, L - PAD : L], e2[:, :], src[:, L - PAD : L])
                nc.vector.scalar_tensor_tensor(
                    out=dst[:, L - PAD : L], in0=coefR_sb[:, :],
                    scalar=src[:, L - 1 : L], in1=dst[:, L - PAD : L],
                    op0=OP.mult, op1=OP.add)

            with ExitStack() as p2stack:
                nch = range(NDC) if phases >= 2 else range(0)
                nfh = range(NFC) if phases >= 2 else range(0)
                p2w = p2stack.enter_context(tc.tile_pool(name="p2w", bufs=1))
                w1_sb = []
                for c in nch:
                    t = p2w.tile([128, DFF], F16, tag=f"w1_{c}")
                    nc.sync.dma_start(out=t[:, :], in_=W1_d[128 * c : 128 * (c + 1), :])
                    w1_sb.append(t)
                w2_sb = []
                for c in nfh:
                    t = p2w.tile([128, D], F16, tag=f"w2_{c}")
                    nc.sync.dma_start(out=t[:, :], in_=W2_d[128 * c : 128 * (c + 1), :])
                    w2_sb.append(t)
                wvo_sb = []
                for c in nch:
                    t = p2w.tile([128, D], F16, tag=f"wvo{c}")
                    nc.sync.dma_start(out=t[:, :], in_=Wvo_d[128 * c : 128 * (c + 1), :])
                    wvo_sb.append(t)

                p2x = p2stack.enter_context(tc.tile_pool(name="p2x", bufs=2))
                p2 = p2stack.enter_context(tc.tile_pool(name="p2", bufs=1))
                ps2 = p2stack.enter_context(tc.tile_pool(name="psum2", bufs=2, space="PSUM"))
                ps2y = p2stack.enter_context(tc.tile_pool(name="psum2y", bufs=1, space="PSUM"))

                # Software-pipelined batch stages. Engines run their queues
                # in order, so emission order decides what independent work a
                # stalled engine has queued ahead: A_b+1's rolls (DVE) are
                # emitted before C_b's z-add, which must wait on B_b's FFN.
                state = {}

                def stage_a(b):
                    # vo' = x @ (Wv Wo); x2 = x + bvo + sum_i w_i roll(vo', d_i);
                    # st = x2 - moving_avg(x2)
                    xt = [p2x.tile([128, L], F16, tag=f"x2t{c}", name=f"x2t{c}")
                          for c in range(NDC)]
                    for c in range(NDC):
                        nc.sync.dma_start(
                            out=xt[c][:, :], in_=xTb_d[b, 128 * c : 128 * (c + 1), :])
                    x2 = [p2.tile([128, L], F16, tag=f"x2_{c}", name=f"x2_{c}", bufs=2)
                          for c in range(NDC)]
                    for c in range(NDC):
                        vo2 = p2.tile([128, 2 * L], F16, tag="vo2", bufs=2)
                        for nb in range(NLB):
                            pv = ps2.tile([128, 512], F32, tag="pv")
                            for cx in range(NDC):
                                nc.tensor.matmul(
                                    pv[:, :],
                                    wvo_sb[cx][:, 128 * c : 128 * (c + 1)],
                                    xt[cx][:, 512 * nb : 512 * (nb + 1)],
                                    start=(cx == 0),
                                    stop=(cx == NDC - 1),
                                )
                            nc.scalar.copy(vo2[:, 512 * nb : 512 * (nb + 1)], pv[:, :])
                            nc.scalar.copy(
                                vo2[:, L + 512 * nb : L + 512 * (nb + 1)], pv[:, :])
                        nc.scalar.activation(
                            x2[c][:, :], xt[c][:, :], AF.Identity,
                            bias=bvoT[:, c : c + 1])
                        for i in range(TOPK):
                            nc.vector.scalar_tensor_tensor(
                                out=x2[c][:, :],
                                in0=vo2[:, bass.ds(dvals[i], L)],
                                scalar=wbc[:, TOPK * b + i : TOPK * b + i + 1],
                                in1=x2[c][:, :],
                                op0=OP.mult,
                                op1=OP.add,
                            )
                    st = [p2.tile([128, L], F16, tag=f"st{c}", name=f"st{c}", bufs=2)
                          for c in range(NDC)]
                    for c in range(NDC):
                        ma_seasonal(p2, st[c], x2[c])
                    state[b] = (x2, st)

                def stage_b(b):
                    # FFN: yf = relu(st W1 + b1) W2, staged out of PSUM via ACT
                    # so stage C's z-add never blocks the DVE on PE progress
                    _, st = state[b]
                    yf = [p2.tile([128, L], F16, tag=f"yf{c}", name=f"yf{c}",
                                  bufs=2) for c in range(NDC)]
                    for nb in range(NLB):
                        lsl = slice(512 * nb, 512 * (nb + 1))
                        py = [ps2y.tile([128, 512], F32, tag=f"py{c}",
                                        name=f"py{c}") for c in range(NDC)]
                        for fc in range(NFC):
                            ph = ps2.tile([128, 512], F32, tag="ph")
                            for c in range(NDC):
                                nc.tensor.matmul(
                                    ph[:, :],
                                    w1_sb[c][:, 128 * fc : 128 * (fc + 1)],
                                    st[c][:, lsl],
                                    start=(c == 0),
                                    stop=(c == NDC - 1),
                                )
                            ht = p2.tile([128, 512], F16, tag="ht", bufs=3)
                            nc.scalar.activation(
                                ht[:, :], ph[:, :], AF.Relu, bias=b1T[:, fc : fc + 1])
                            for c in range(NDC):
                                nc.tensor.matmul(
                                    py[c][:, :],
                                    w2_sb[fc][:, 128 * c : 128 * (c + 1)],
                                    ht[:, :],
                                    start=(fc == 0),
                                    stop=(fc == NFC - 1),
                                )
                        for c in range(NDC):
                            nc.scalar.copy(yf[c][:, lsl], py[c][:, :])
                    state[b] = state[b] + (yf,)

                def stage_c(b):
                    # z = st + yf + b2; res = z - moving_avg(z) -> resT
                    x2, st, yf = state.pop(b)
                    z = x2  # reuse buffers
                    for c in range(NDC):
                        nc.vector.scalar_tensor_tensor(
                            out=z[c][:, :], in0=yf[c][:, :],
                            scalar=b2T[:, c : c + 1], in1=st[c][:, :],
                            op0=OP.add, op1=OP.add)
                    for c in range(NDC):
                        rt = p2.tile([128, L], F32, tag="rt", bufs=2)
                        ma_seasonal(p2, rt, z[c])
                        nc.sync.dma_start(
                            out=resT[b, 128 * c : 128 * (c + 1), :], in_=rt[:, :])

                if phases >= 2:
                    plan = [(stage_a, 0), (stage_a, 1), (stage_b, 0), (stage_c, 0),
                            (stage_a, 2), (stage_b, 1), (stage_c, 1),
                            (stage_a, 3), (stage_b, 2), (stage_c, 2),
                            (stage_b, 3), (stage_c, 3)]
                    for fn, b in plan:
                        fn(b)

    nc.compile()
    return nc


_CACHE = {}


def _get_nc(dbg=False):
    if dbg not in _CACHE:
        _CACHE[dbg] = build(dbg=dbg)
    return _CACHE[dbg]


def make_in_maps(x, Wq, bq, Wk, bk, Wv, bv, Wo, bo, W1, b1, W2, b2):
    coefL_np, coefR_np = _host_consts()
    x = np.asarray(x, np.float32)
    Wkq = (np.asarray(Wk, np.float64) @ np.asarray(Wq, np.float64).T).astype(np.float32)
    Wvo = (np.asarray(Wv, np.float64) @ np.asarray(Wo, np.float64)).astype(np.float32)
    bvo = (np.asarray(bv, np.float64) @ np.asarray(Wo, np.float64)
           + np.asarray(bo, np.float64)).astype(np.float32)
    shared = {
        "Wkq": Wkq,
        "Wvo": Wvo.astype(np.float16),
        "W1": np.ascontiguousarray(np.asarray(W1, np.float16)),
        "W2": np.ascontiguousarray(np.asarray(W2, np.float16)),
        "bvo": np.ascontiguousarray(bvo.reshape(NDC, 128).T),
        "b1": np.ascontiguousarray(np.asarray(b1, np.float32).reshape(NFC, 128).T),
        "b2": np.ascontiguousarray(np.asarray(b2, np.float32).reshape(NDC, 128).T),
        "coefL": coefL_np,
        "coefR": coefR_np,
    }
    in_maps = []
    for c in range(N_CORES):
        xs = x[BC * c : BC * (c + 1)]
        xsT = np.ascontiguousarray(xs.transpose(0, 2, 1))
        in_maps.append({**shared, "xT": xsT, "xTb": xsT.astype(np.float16)})
    return in_maps


def run(inputs, dbg=False, trace=False):
    nc = _get_nc(dbg=dbg)
    in_maps = make_in_maps(**inputs)
    res = run_bass_kernel_spmd(
        nc, in_maps, core_ids=list(range(N_CORES)), trace=trace)
    out = np.empty((B, L, D), np.float32)
    for c in range(N_CORES):
        rt = res.results[c]["resT"]  # [BC, D, L]
        out[BC * c : BC * (c + 1)] = rt.transpose(0, 2, 1)
    return out, res


def kernel(**inputs):
    out, _ = run(inputs)
    return out


_NULL_CACHE = {}


def _get_null_nc():
    if "nc" not in _NULL_CACHE:
        nc = bacc.Bacc("TRN2", target_bir_lowering=False, debug=False,
                       num_devices=N_CORES)
        ins = {
            "xT": [BC, D, L, F32], "xTb": [BC, D, L, F16],
            "Wkq": [D, D, F32], "Wvo": [D, D, F16],
            "W1": [D, DFF, F16], "W2": [DFF, D, F16],
            "bvo": [128, NDC, F32], "b1": [128, NFC, F32], "b2": [128, NDC, F32],
            "coefL": [128, PAD, F32], "coefR": [128, PAD, F32],
        }
        for name, shape in ins.items():
            nc.dram_tensor(name, shape[:-1], shape[-1], kind="ExternalInput")
        resT = nc.dram_tensor("resT", [BC, D, L], F32, kind="ExternalOutput")
        with tile.TileContext(nc) as tc:
            with tc.tile_pool(name="sb", bufs=1) as sb:
                t = sb.tile([128, 4], F32, name="t")
                nc.vector.memset(t[:, :], 0.0)
                nc.sync.dma_start(out=resT[0, 0:128, 0:4], in_=t[:, :])
        nc.compile()
        _NULL_CACHE["nc"] = nc
    return _NULL_CACHE["nc"]


def time_null(inputs, reps=3):
    import time as _time
    nc = _get_null_nc()
    in_maps = make_in_maps(**inputs)
    run_bass_kernel_spmd(nc, in_maps, core_ids=list(range(N_CORES)))
    best = float("inf")
    for _ in range(reps):
        t0 = _time.time()
        run_bass_kernel_spmd(nc, in_maps, core_ids=list(range(N_CORES)))
        best = min(best, _time.time() - t0)
    return best


def time_main(inputs, reps=3, dbg=False):
    import time as _time
    nc = _get_nc(dbg=dbg)
    in_maps = make_in_maps(**inputs)
    run_bass_kernel_spmd(nc, in_maps, core_ids=list(range(N_CORES)))
    best = float("inf")
    for _ in range(reps):
        t0 = _time.time()
        run_bass_kernel_spmd(nc, in_maps, core_ids=list(range(N_CORES)))
        best = min(best, _time.time() - t0)
    return best

